# revision 9
# baseline (speedup 1.0000x reference)
"""Trainium2 Bass kernel for nn_BreakthroughSNN (predictive-coding SNN).

Structure (derived analytically from the reference; validated bit-exact vs a
numpy golden model in fp32):

- The (t, position) recurrence is strictly sequential: 4 time steps x 128
  positions, with 4 dependent (B,1024)x(1024,1024) matmuls per position on
  the critical path (inf0, inf1, genD1, genD0).
- Off the critical path: the encoder projection (emb @ enc_w.T) is position-
  independent and shared across all 4 time steps (computed once, batched);
  the upward gen inputs state_j @ gen_w_j.T use the *previous* time step's
  states, so they are batched per time step.  genD1's output doubles as next
  step's upward gen1 input (identical product), so only gen0 needs a batched
  recompute per time step.
- The downward pass has err = relu(-spike) = 0, so its inf matmuls vanish
  and mem_inf just decays (biases are zero: verified at runtime, fast path).

Distribution: data-parallel over batch, 4 samples per core on 8 cores.  All
weights and state are SBUF-resident in bf16 (bf16 shifts the spikes output
by ~6e-5 relative; logits are exactly zero for this model: the final
downward spike never fires, verified robust to >=4e-3 weight perturbation).

Layout: feature-major "A-tiles" (128 partitions = feature%128, columns =
4*chunk + sample), so matmul outputs (o-strip, batch) land directly in the
layout the next matmul's moving operand needs -- no transposes anywhere.
"""

import os
import sys

import numpy as np
import ml_dtypes

for _p in ("/opt/trn_rl_repo",):
    if _p not in sys.path:
        sys.path.insert(0, _p)

import concourse.bass as bass
import concourse.mybir as mybir
import concourse.tile as tile
from concourse import bacc
from concourse.alu_op_type import AluOpType as ALU
from concourse.bass_utils import run_bass_kernel_spmd


def _install_ntff_hook_shim():
    """The image's antenv package lacks axon_hooks; synthesize it and register
    the ctypes NTFF hook so trace=True / BASS_TRACE=1 can profile."""
    import types
    if "antenv.axon_hooks" in sys.modules:
        return
    try:
        mod = types.ModuleType("antenv.axon_hooks")
        state = {"hook": None}
        mod.set_axon_ntff_profile_hook = lambda h: state.update(hook=h)
        mod.get_axon_ntff_profile_hook = lambda: state["hook"]
        from trn_agent_boot.trn_boot import _ntff_profile_via_ctypes
        mod.set_axon_ntff_profile_hook(
            _ntff_profile_via_ctypes("/opt/axon/libaxon_pjrt.so"))
        sys.modules["antenv.axon_hooks"] = mod
        import antenv
        antenv.axon_hooks = mod
    except Exception:
        pass


_install_ntff_hook_shim()

BF16 = mybir.dt.bfloat16
F32 = mybir.dt.float32

B, S, V, DM, L, T = 32, 128, 32000, 1024, 2, 4
NCORE = 8
BC = B // NCORE                     # batch per core
AC = 4 * 8                          # A-tile cols per position (8 chunks x 4)
KC = DM // 128                      # feature chunks
DECAY = float(np.exp(-1.0 / 2.0))
ACT_RELU = mybir.ActivationFunctionType.Relu

last_exec_time_ns = None            # set by kernel() when BASS_TRACE=1
_prog_cache = {}


def _bf16(x):
    return np.asarray(x).astype(ml_dtypes.bfloat16)


def _pack_w(w_t):
    """W' (K=1024, O=1024) -> (128, 8192) bf16; lhsT tile (kc, oc) is
    [:, kc*1024 + oc*128 : kc*1024 + (oc+1)*128]."""
    arr = np.ascontiguousarray(w_t).reshape(KC, 128, DM)
    return _bf16(arr.transpose(1, 0, 2).reshape(128, KC * DM))


def _pack_x(rows):
    """rows (BC, n_pos, DM) fp32 -> (128, KC*n_pos*BC) bf16, col = kc*n_pos*BC + i*BC + b."""
    n_pos = rows.shape[1]
    arr = np.ascontiguousarray(rows.transpose(2, 1, 0))  # (f, i, b)
    arr = arr.reshape(KC, 128, n_pos * BC)               # (kc, p, i*b)
    return _bf16(np.ascontiguousarray(arr.transpose(1, 0, 2)).reshape(128, KC * n_pos * BC))


def _build_scan_program(n_pos=S, t_steps=T):
    nc = bacc.Bacc(None, target_bir_lowering=False)

    xemb_d = nc.dram_tensor("xemb", [128, KC * n_pos * BC], BF16, kind="ExternalInput")
    w_d = {}
    for name in ("wenc", "wg0", "wg1", "wi0", "wi1"):
        w_d[name] = nc.dram_tensor(name, [128, KC * DM], BF16, kind="ExternalInput")
    facc_d = nc.dram_tensor("facc", [128, AC], F32, kind="ExternalOutput")
    errsum_d = nc.dram_tensor("errsum", [1, 1], F32, kind="ExternalOutput")

    with tile.TileContext(nc) as tc:
        with (
            tc.tile_pool(name="big", bufs=1) as big,
            tc.tile_pool(name="tmp", bufs=3) as tmp,
            tc.tile_pool(name="ps", bufs=8, space="PSUM") as ps,
        ):
            # persistent SBUF state
            W = {k: big.tile([128, KC * DM], BF16, tag=k, name="W" + k) for k in w_d}
            XE = big.tile([128, KC * n_pos * BC], BF16, tag="xe")
            EIN = big.tile([128, n_pos * AC], F32, tag="ein")
            G01 = big.tile([128, n_pos * 2 * AC], F32, tag="g01")
            S0 = big.tile([128, n_pos * AC], BF16, tag="s0")
            S1 = big.tile([128, n_pos * AC], BF16, tag="s1")
            ME = big.tile([128, AC], F32, tag="me")
            MG = big.tile([128, 2 * AC], F32, tag="mg")
            MI = big.tile([128, 2 * AC], F32, tag="mi")
            FACC = big.tile([128, AC], F32, tag="facc")
            EACC = big.tile([128, 1], F32, tag="eacc")
            ESUM = big.tile([128, 2 * n_pos], F32, tag="esum")
            ONES = big.tile([128, 1], F32, tag="ones")
            ERRT = big.tile([1, 1], F32, tag="errt")

            for k, d in w_d.items():
                nc.sync.dma_start(W[k][:], d[:])
            nc.sync.dma_start(XE[:], xemb_d[:])

            for t_ in (ME, MG, MI, FACC, EACC, S0, S1, G01):
                nc.vector.memset(t_[:], 0.0)
            nc.vector.memset(ONES[:], 1.0)

            def mm_1024(psum_ap, wt, rhs_of_kc, n_out_chunks=8, oc_cols=4):
                """out[oc_block] += sum_kc  W(kc,oc).T @ rhs(kc)."""
                for oc in range(n_out_chunks):
                    for kc in range(KC):
                        nc.tensor.matmul(
                            psum_ap[:, oc * oc_cols:(oc + 1) * oc_cols],
                            wt[:, kc * DM + oc * 128: kc * DM + (oc + 1) * 128],
                            rhs_of_kc(kc),
                            start=(kc == 0),
                            stop=(kc == KC - 1),
                        )

            # ---- Phase 1: EIN = emb_rows @ enc_w.T (shared across t) ----
            npc = n_pos * BC
            ein4 = EIN[:].rearrange("p (i c b) -> p i c b", c=KC, b=BC)
            for oc in range(KC):
                pz = ps.tile([128, npc], F32, tag="ps")
                for kc in range(KC):
                    nc.tensor.matmul(
                        pz[:],
                        W["wenc"][:, kc * DM + oc * 128: kc * DM + (oc + 1) * 128],
                        XE[:, kc * npc:(kc + 1) * npc],
                        start=(kc == 0),
                        stop=(kc == KC - 1),
                    )
                nc.scalar.copy(ein4[:, :, oc, :], pz[:].rearrange("p (i b) -> p i b", b=BC))

            # ---- Phase 2: the sequential scan ----
            s04 = S0[:].rearrange("p (i c b) -> p i c b", c=KC, b=BC)
            g015 = G01[:].rearrange("p (i h c b) -> p i h c b", h=2, c=KC, b=BC)

            def pos_body(i):
                ein_i = EIN[:, AC * i: AC * (i + 1)]
                g01_i = G01[:, 2 * AC * i: 2 * AC * (i + 1)]
                s0_i = S0[:, AC * i: AC * (i + 1)]
                s1_i = S1[:, AC * i: AC * (i + 1)]
                g1up_i = G01[:, 2 * AC * i + AC: 2 * AC * (i + 1)]

                bu = tmp.tile([128, AC], BF16, tag="bu")
                kp = tmp.tile([128, 2 * AC], BF16, tag="kp")
                err0 = tmp.tile([128, AC], BF16, tag="err0")
                t1 = tmp.tile([128, AC], BF16, tag="t1")
                err1 = tmp.tile([128, AC], BF16, tag="err1")
                pd1 = tmp.tile([128, AC], BF16, tag="pd1")

                # upward elementwise (encoder + both gen LIFs)
                nc.vector.scalar_tensor_tensor(ME[:], ME[:], DECAY, ein_i, ALU.mult, ALU.add)
                nc.vector.tensor_scalar(bu[:], ME[:], 1.0, None, ALU.is_ge)
                nc.vector.scalar_tensor_tensor(ME[:], ME[:], 1.0, ME[:], ALU.is_lt, ALU.mult)
                nc.vector.scalar_tensor_tensor(MG[:], MG[:], DECAY, g01_i, ALU.mult, ALU.add)
                nc.vector.tensor_scalar(kp[:], MG[:], 1.0, None, ALU.is_lt)
                nc.vector.scalar_tensor_tensor(MG[:], MG[:], 1.0, MG[:], ALU.is_lt, ALU.mult)
                nc.vector.scalar_tensor_tensor(
                    err0[:], bu[:], 1.0, kp[:, 0:AC], ALU.bypass, ALU.mult,
                    accum_out=ESUM[:, 2 * i: 2 * i + 1])

                # MM1: z0 = err0 @ inf_w0.T  -> inf0 LIF -> ns0
                zp0 = ps.tile([128, AC], F32, tag="ps")
                mm_1024(zp0[:], W["wi0"], lambda kc: err0[:, BC * kc: BC * (kc + 1)])
                mi0, mi1 = MI[:, 0:AC], MI[:, AC:2 * AC]
                nc.vector.scalar_tensor_tensor(mi0, mi0, DECAY, zp0[:], ALU.mult, ALU.add)
                nc.vector.scalar_tensor_tensor(s0_i, mi0, 1.0, s0_i, ALU.is_ge, ALU.add)
                nc.vector.scalar_tensor_tensor(mi0, mi0, 1.0, mi0, ALU.is_lt, ALU.mult)
                nc.vector.scalar_tensor_tensor(
                    t1[:], kp[:, AC:2 * AC], -1.0, s0_i, ALU.add, ALU.add)
                nc.vector.tensor_scalar(
                    err1[:], t1[:], 0.0, None, ALU.max, ALU.add,
                    accum_out=ESUM[:, 2 * i + 1: 2 * i + 2])

                # MM2: z1 = err1 @ inf_w1.T -> inf1 LIF -> ns1
                zp1 = ps.tile([128, AC], F32, tag="ps")
                mm_1024(zp1[:], W["wi1"], lambda kc: err1[:, BC * kc: BC * (kc + 1)])
                nc.vector.scalar_tensor_tensor(mi1, mi1, DECAY, zp1[:], ALU.mult, ALU.add)
                nc.vector.scalar_tensor_tensor(s1_i, mi1, 1.0, s1_i, ALU.is_ge, ALU.add)
                nc.vector.scalar_tensor_tensor(mi1, mi1, 1.0, mi1, ALU.is_lt, ALU.mult)

                # MM3: w1d = ns1 @ gen_w1.T (dual use: next t's g1up)
                w1d = ps.tile([128, AC], F32, tag="ps")
                mm_1024(w1d[:], W["wg1"], lambda kc: s1_i[:, BC * kc: BC * (kc + 1)])
                nc.scalar.copy(g1up_i, w1d[:])
                mg0, mg1 = MG[:, 0:AC], MG[:, AC:2 * AC]
                nc.vector.scalar_tensor_tensor(mg1, mg1, DECAY, w1d[:], ALU.mult, ALU.add)
                nc.vector.tensor_scalar(pd1[:], mg1, 1.0, None, ALU.is_ge)
                nc.vector.scalar_tensor_tensor(mg1, mg1, 1.0, mg1, ALU.is_lt, ALU.mult)

                # MM4: w0d = pd1 @ gen_w0.T
                w0d = ps.tile([128, AC], F32, tag="ps")
                mm_1024(w0d[:], W["wg0"], lambda kc: pd1[:, BC * kc: BC * (kc + 1)])
                nc.vector.scalar_tensor_tensor(mg0, mg0, DECAY, w0d[:], ALU.mult, ALU.add)
                if i == n_pos - 1:
                    nc.vector.scalar_tensor_tensor(
                        FACC[:], mg0, 1.0, FACC[:], ALU.is_ge, ALU.add)
                nc.vector.scalar_tensor_tensor(mg0, mg0, 1.0, mg0, ALU.is_lt, ALU.mult)

                # downward inf LIFs reduce to pure decay (zero bias, no spikes)
                nc.vector.tensor_scalar_mul(MI[:], MI[:], DECAY)

            with tc.For_i(0, t_steps, 1) as _t:
                for i in range(n_pos):
                    pos_body(i)

                # fold per-position err sums
                tred = tmp.tile([128, 1], F32, tag="tred")
                nc.vector.tensor_reduce(tred[:], ESUM[:], mybir.AxisListType.X, ALU.add)
                nc.vector.tensor_add(EACC[:], EACC[:], tred[:])

                # batched g0up recompute for next t: state0 @ gen_w0.T
                for oc in range(KC):
                    pz = ps.tile([128, n_pos * BC], F32, tag="ps")
                    for kc in range(KC):
                        nc.tensor.matmul(
                            pz[:],
                            W["wg0"][:, kc * DM + oc * 128: kc * DM + (oc + 1) * 128],
                            s04[:, :, kc, :],
                            start=(kc == 0),
                            stop=(kc == KC - 1),
                        )
                    nc.scalar.copy(
                        g015[:, :, 0, oc, :],
                        pz[:].rearrange("p (i b) -> p i b", b=BC))

            # ---- Phase 3: outputs ----
            nc.sync.dma_start(facc_d[:], FACC[:])
            pe = ps.tile([1, 1], F32, tag="ps")
            nc.tensor.matmul(pe[:], EACC[:], ONES[:], start=True, stop=True)
            nc.vector.tensor_copy(ERRT[:], pe[:])
            nc.sync.dma_start(errsum_d[:], ERRT[:])

    nc.compile()
    return nc


def _build_logits_program():
    """General path (unused when final_act == 0): logits strip per core.
    Inputs: x32 (128, 32*KC) bf16 = final_act.T feature-major (col=32*kc+b);
    owt (128, KC*4096) bf16 = padded out_w.T strip tiles. Output (32, 4096)."""
    nc = bacc.Bacc(None, target_bir_lowering=False)
    OV = 4096
    x_d = nc.dram_tensor("x32", [128, 32 * KC], BF16, kind="ExternalInput")
    w_d = nc.dram_tensor("owt", [128, KC * OV], BF16, kind="ExternalInput")
    o_d = nc.dram_tensor("lg", [32, OV], F32, kind="ExternalOutput")
    with tile.TileContext(nc) as tc:
        with (
            tc.tile_pool(name="p", bufs=1) as pool,
            tc.tile_pool(name="ps", bufs=8, space="PSUM") as ps,
        ):
            X = pool.tile([128, 32 * KC], BF16, tag="x")
            WT = pool.tile([128, KC * OV], BF16, tag="w")
            OUT = pool.tile([32, OV], F32, tag="o")
            nc.sync.dma_start(X[:], x_d[:])
            nc.sync.dma_start(WT[:], w_d[:])
            for ob in range(OV // 512):
                pz = ps.tile([32, 512], F32, tag="ps")
                for kc in range(KC):
                    nc.tensor.matmul(
                        pz[:],
                        X[:, 32 * kc: 32 * (kc + 1)],
                        WT[:, kc * OV + ob * 512: kc * OV + (ob + 1) * 512],
                        start=(kc == 0),
                        stop=(kc == KC - 1),
                    )
                nc.vector.tensor_copy(OUT[:, ob * 512:(ob + 1) * 512], pz[:])
            nc.sync.dma_start(o_d[:], OUT[:])
    nc.compile()
    return nc


def _scan_inputs_for_core(c, input_ids, emb, enc_w, gen_w, inf_w):
    ids = np.asarray(input_ids)[c * BC:(c + 1) * BC, :]        # (BC, S)
    rows = np.asarray(emb, np.float32)[ids]                    # (BC, S, DM)
    return {
        "xemb": _pack_x(rows),
        "wenc": _pack_w(np.asarray(enc_w, np.float32).T),
        "wg0": _pack_w(np.asarray(gen_w[0], np.float32).T),
        "wg1": _pack_w(np.asarray(gen_w[1], np.float32).T),
        "wi0": _pack_w(np.asarray(inf_w[0], np.float32).T),
        "wi1": _pack_w(np.asarray(inf_w[1], np.float32).T),
    }


def kernel(input_ids, emb, enc_w, enc_b, gen_w, gen_b, inf_w, inf_b, out_w, out_b):
    global last_exec_time_ns
    ids_dtype = np.asarray(input_ids).dtype

    zero_bias = (not np.any(enc_b)) and (not np.any(gen_b)) and (not np.any(inf_b))
    assert zero_bias, "kernel implements the zero-bias fast path only"

    if "scan" not in _prog_cache:
        _prog_cache["scan"] = _build_scan_program()
    nc = _prog_cache["scan"]

    in_maps = [
        _scan_inputs_for_core(c, input_ids, emb, enc_w, gen_w, inf_w)
        for c in range(NCORE)
    ]
    res = run_bass_kernel_spmd(nc, in_maps, core_ids=list(range(NCORE)))
    last_exec_time_ns = res.exec_time_ns

    fa = []
    err_tot = 0.0
    for c in range(NCORE):
        f = res.results[c]["facc"]                              # (128, AC)
        fa.append(f.reshape(128, KC, BC).transpose(2, 1, 0).reshape(BC, DM))
        err_tot += float(res.results[c]["errsum"][0, 0])
    final_act = (np.concatenate(fa, axis=0) / T).astype(np.float32)   # (B, DM)
    spikes = np.float32(err_tot / (T * S * B))

    out_b = np.asarray(out_b, np.float32)
    if not np.any(final_act):
        # exact algebraic shortcut: 0 @ out_w.T == 0
        logits = np.broadcast_to(out_b, (B, V)).copy().astype(np.float32)
    else:
        logits = _logits_on_device(final_act, out_w, out_b)
    _ = ids_dtype
    return logits, spikes


def _logits_on_device(final_act, out_w, out_b):
    if "logits" not in _prog_cache:
        _prog_cache["logits"] = _build_logits_program()
    nc = _prog_cache["logits"]
    OV = 4096
    # x32: col = 32*kc + b  <-  final_act[b, 128*kc + p]
    arr = final_act.reshape(B, KC, 128).transpose(2, 1, 0)      # (p, kc, b)
    x32 = _bf16(np.ascontiguousarray(arr).reshape(128, KC * B))
    wt = np.asarray(out_w, np.float32)                          # (V, DM)
    in_maps = []
    for c in range(NCORE):
        strip = np.zeros((OV, DM), np.float32)
        lo, hi = c * (V // NCORE), (c + 1) * (V // NCORE)
        strip[: hi - lo] = wt[lo:hi]
        arrw = np.ascontiguousarray(strip.T).reshape(KC, 128, OV)
        wpack = _bf16(arrw.transpose(1, 0, 2).reshape(128, KC * OV))
        in_maps.append({"x32": x32, "owt": wpack})
    res = run_bass_kernel_spmd(nc, in_maps, core_ids=list(range(NCORE)))
    logits = np.concatenate(
        [res.results[c]["lg"][:, : V // NCORE] for c in range(NCORE)], axis=1)
    return (logits + out_b).astype(np.float32)


# revision 21
# speedup vs baseline: 1.0522x; 1.0522x over previous
"""Trainium2 Bass kernel for nn_BreakthroughSNN (predictive-coding SNN).

Structure (derived analytically from the reference; validated bit-exact vs a
numpy golden model in fp32):

- The (t, position) recurrence is strictly sequential: 4 time steps x 128
  positions, with 4 dependent (B,1024)x(1024,1024) matmuls per position on
  the critical path (inf0, inf1, genD1, genD0).
- Off the critical path: the encoder projection (emb @ enc_w.T) is position-
  independent and shared across all 4 time steps (computed once, batched);
  the upward gen inputs state_j @ gen_w_j.T use the *previous* time step's
  states, so they are batched per time step.  genD1's output doubles as next
  step's upward gen1 input (identical product), so only gen0 needs a batched
  recompute per time step.
- The downward pass has err = relu(-spike) = 0, so its inf matmuls vanish
  and mem_inf just decays (biases are zero: verified at runtime, fast path).

Distribution: data-parallel over batch, 4 samples per core on 8 cores.  All
weights and state are SBUF-resident in bf16 (bf16 shifts the spikes output
by ~6e-5 relative; logits are exactly zero for this model: the final
downward spike never fires, verified robust to >=4e-3 weight perturbation).

Layout: feature-major "A-tiles" (128 partitions = feature%128, columns =
4*chunk + sample), so matmul outputs (o-strip, batch) land directly in the
layout the next matmul's moving operand needs -- no transposes anywhere.
"""

import os
import sys

import numpy as np
import ml_dtypes

for _p in ("/opt/trn_rl_repo",):
    if _p not in sys.path:
        sys.path.insert(0, _p)

import concourse.bass as bass
import concourse.mybir as mybir
import concourse.tile as tile
from concourse import bacc
from concourse.alu_op_type import AluOpType as ALU
from concourse.bass_utils import run_bass_kernel_spmd


def _install_ntff_hook_shim():
    """The image's antenv package lacks axon_hooks; synthesize it and register
    the ctypes NTFF hook so trace=True / BASS_TRACE=1 can profile."""
    import types
    if "antenv.axon_hooks" in sys.modules:
        return
    try:
        mod = types.ModuleType("antenv.axon_hooks")
        state = {"hook": None}
        mod.set_axon_ntff_profile_hook = lambda h: state.update(hook=h)
        mod.get_axon_ntff_profile_hook = lambda: state["hook"]
        from trn_agent_boot.trn_boot import _ntff_profile_via_ctypes
        mod.set_axon_ntff_profile_hook(
            _ntff_profile_via_ctypes("/opt/axon/libaxon_pjrt.so"))
        sys.modules["antenv.axon_hooks"] = mod
        import antenv
        antenv.axon_hooks = mod
    except Exception:
        pass


_install_ntff_hook_shim()

BF16 = mybir.dt.bfloat16
F32 = mybir.dt.float32
FP8 = mybir.dt.float8e4
FP8_WEIGHTS = True                  # scan weights in fp8 e4m3 (spikes rel err 2.6e-4)

B, S, V, DM, L, T = 32, 128, 32000, 1024, 2, 4
NCORE = 8
BC = B // NCORE                     # batch per core
AC = 4 * 8                          # A-tile cols per position (8 chunks x 4)
KC = DM // 128                      # feature chunks
DECAY = float(np.exp(-1.0 / 2.0))
DECAY2 = float(np.float32(DECAY) * np.float32(DECAY))
INVD = float(np.float32(1.0) / np.float32(DECAY))
ACT_COPY = mybir.ActivationFunctionType.Copy

last_exec_time_ns = None            # set by kernel() when BASS_TRACE=1
_prog_cache = {}


def _bf16(x):
    return np.asarray(x).astype(ml_dtypes.bfloat16)


def _pack_w(w_t, np_dtype=ml_dtypes.bfloat16):
    """W' (K=1024, O=1024) -> (128, 8192); lhsT tile (kc, oc) is
    [:, kc*1024 + oc*128 : kc*1024 + (oc+1)*128]."""
    arr = np.ascontiguousarray(w_t).reshape(KC, 128, DM)
    return np.ascontiguousarray(arr.transpose(1, 0, 2)).reshape(128, KC * DM).astype(np_dtype)


def _pack_x(rows):
    """rows (BC, n_pos, DM) fp32 -> (128, KC*n_pos*BC) bf16, col = kc*n_pos*BC + i*BC + b."""
    n_pos = rows.shape[1]
    arr = np.ascontiguousarray(rows.transpose(2, 1, 0))  # (f, i, b)
    arr = arr.reshape(KC, 128, n_pos * BC)               # (kc, p, i*b)
    return _bf16(np.ascontiguousarray(arr.transpose(1, 0, 2)).reshape(128, KC * n_pos * BC))


def _build_scan_program(n_pos=S, t_steps=T):
    nc = bacc.Bacc(None, target_bir_lowering=False)

    wdt = FP8 if FP8_WEIGHTS else BF16
    xemb_d = nc.dram_tensor("xemb", [128, KC * n_pos * BC], BF16, kind="ExternalInput")
    w_d = {}
    for name in ("wenc", "wg0", "wg1", "wi0", "wi1"):
        w_d[name] = nc.dram_tensor(name, [128, KC * DM], BF16 if name == "wenc" else wdt,
                                   kind="ExternalInput")
    facc_d = nc.dram_tensor("facc", [128, AC], F32, kind="ExternalOutput")
    errsum_d = nc.dram_tensor("errsum", [1, 1], F32, kind="ExternalOutput")

    with tile.TileContext(nc) as tc:
        with (
            tc.tile_pool(name="big", bufs=1) as big,
            tc.tile_pool(name="tmp", bufs=3) as tmp,
            tc.tile_pool(name="ps", bufs=8, space="PSUM") as ps,
        ):
            # persistent SBUF state
            W = {k: big.tile([128, KC * DM], BF16 if k == "wenc" else wdt,
                             tag=k, name="W" + k) for k in w_d}
            XE = big.tile([128, KC * n_pos * BC], BF16, tag="xe")
            EIN = big.tile([128, n_pos * AC], F32, tag="ein")
            G01 = big.tile([128, n_pos * 2 * AC], F32, tag="g01")
            S0 = big.tile([128, n_pos * AC], BF16, tag="s0")
            S1 = big.tile([128, n_pos * AC], BF16, tag="s1")
            ME = big.tile([128, AC], F32, tag="me")
            MG = big.tile([128, 2 * AC], F32, tag="mg")
            MI = big.tile([128, 2 * AC], F32, tag="mi")
            MGK = big.tile([128, 2 * AC], F32, tag="mgk")
            KP1 = big.tile([128, AC], BF16, tag="kp1")
            PRE1 = big.tile([128, AC], BF16, tag="pre1")
            C0 = big.tile([128, AC], F32, tag="c0")
            FACC = big.tile([128, AC], F32, tag="facc")
            EACC = big.tile([128, 1], F32, tag="eacc")
            ESUM = big.tile([128, 2 * n_pos], F32, tag="esum")
            ONES = big.tile([128, 1], F32, tag="ones")
            ERRT = big.tile([1, 1], F32, tag="errt")

            for k, d in w_d.items():
                nc.sync.dma_start(W[k][:], d[:])
            nc.sync.dma_start(XE[:], xemb_d[:])

            for t_ in (ME, MG, MI, MGK, FACC, EACC, S0, S1, G01):
                nc.vector.memset(t_[:], 0.0)
            nc.vector.memset(ONES[:], 1.0)

            def mm_1024(psum_ap, wt, rhs_of_kc, n_out_chunks=8, oc_cols=4):
                """out[oc_block] += sum_kc  W(kc,oc).T @ rhs(kc)."""
                for oc in range(n_out_chunks):
                    for kc in range(KC):
                        nc.tensor.matmul(
                            psum_ap[:, oc * oc_cols:(oc + 1) * oc_cols],
                            wt[:, kc * DM + oc * 128: kc * DM + (oc + 1) * 128],
                            rhs_of_kc(kc),
                            start=(kc == 0),
                            stop=(kc == KC - 1),
                        )

            # ---- Phase 1: EIN = emb_rows @ enc_w.T (shared across t) ----
            npc = n_pos * BC
            ein4 = EIN[:].rearrange("p (i c b) -> p i c b", c=KC, b=BC)
            for oc in range(KC):
                pz = ps.tile([128, npc], F32, tag="ps")
                for kc in range(KC):
                    nc.tensor.matmul(
                        pz[:],
                        W["wenc"][:, kc * DM + oc * 128: kc * DM + (oc + 1) * 128],
                        XE[:, kc * npc:(kc + 1) * npc],
                        start=(kc == 0),
                        stop=(kc == KC - 1),
                    )
                nc.scalar.copy(ein4[:, :, oc, :], pz[:].rearrange("p (i b) -> p i b", b=BC))

            # ---- Phase 2: the sequential scan ----
            s04 = S0[:].rearrange("p (i c b) -> p i c b", c=KC, b=BC)
            g015 = G01[:].rearrange("p (i h c b) -> p i h c b", h=2, c=KC, b=BC)
            mi0, mi1 = MI[:, 0:AC], MI[:, AC:2 * AC]
            mg0, mg1 = MG[:, 0:AC], MG[:, AC:2 * AC]

            def up_joint(j):
                """Off-chain gen up-integrate/reset for position j, plus the
                derived kp1/pre1/c0 used by position j's critical chain."""
                g01_j = G01[:, 2 * AC * j: 2 * AC * (j + 1)]
                s0_j = S0[:, AC * j: AC * (j + 1)]
                nc.vector.scalar_tensor_tensor(MG[:], MGK[:], DECAY, g01_j, ALU.mult, ALU.add)
                nc.vector.tensor_scalar(KP1[:], MG[:, AC:2 * AC], 1.0, None, ALU.is_lt)
                nc.vector.scalar_tensor_tensor(MG[:], MG[:], 1.0, MG[:], ALU.is_lt, ALU.mult)
                nc.vector.scalar_tensor_tensor(
                    PRE1[:], KP1[:], -1.0, s0_j, ALU.add, ALU.add)
                nc.scalar.activation(C0[:], G01[:, 2 * AC * j: 2 * AC * j + AC],
                                     ACT_COPY, bias=INVD, scale=-INVD)

            def pos_body(i):
                ein_j = EIN[:, AC * ((i + 1) % n_pos): AC * ((i + 1) % n_pos) + AC]
                s0_i = S0[:, AC * i: AC * (i + 1)]
                s1_i = S1[:, AC * i: AC * (i + 1)]
                g1up_i = G01[:, 2 * AC * i + AC: 2 * AC * (i + 1)]

                kp0 = tmp.tile([128, AC], BF16, tag="kp0")
                err0 = tmp.tile([128, AC], BF16, tag="err0")
                t1 = tmp.tile([128, AC], BF16, tag="t1")
                err1 = tmp.tile([128, AC], BF16, tag="err1")
                pd1 = tmp.tile([128, AC], BF16, tag="pd1")

                # ---- critical chain ----
                nc.vector.scalar_tensor_tensor(kp0[:], MGK[:, 0:AC], 0.0, C0[:],
                                               ALU.bypass, ALU.is_lt)
                nc.vector.scalar_tensor_tensor(
                    err0[:], ME[:], 1.0, kp0[:], ALU.is_ge, ALU.mult,
                    accum_out=ESUM[:, 2 * i: 2 * i + 1])
                zp0 = ps.tile([128, AC], F32, tag="ps")
                mm_1024(zp0[:], W["wi0"], lambda kc: err0[:, BC * kc: BC * (kc + 1)])
                nc.vector.scalar_tensor_tensor(mi0, mi0, DECAY2, zp0[:], ALU.mult, ALU.add)
                nc.vector.scalar_tensor_tensor(t1[:], mi0, 1.0, PRE1[:], ALU.is_ge, ALU.add)
                nc.vector.tensor_scalar(
                    err1[:], t1[:], 0.0, None, ALU.max, ALU.add,
                    accum_out=ESUM[:, 2 * i + 1: 2 * i + 2])
                zp1 = ps.tile([128, AC], F32, tag="ps")
                mm_1024(zp1[:], W["wi1"], lambda kc: err1[:, BC * kc: BC * (kc + 1)])
                nc.vector.scalar_tensor_tensor(mi1, mi1, DECAY2, zp1[:], ALU.mult, ALU.add)
                nc.vector.scalar_tensor_tensor(s1_i, mi1, 1.0, s1_i, ALU.is_ge, ALU.add)
                w1d = ps.tile([128, AC], F32, tag="ps")
                mm_1024(w1d[:], W["wg1"], lambda kc: s1_i[:, BC * kc: BC * (kc + 1)])
                nc.vector.scalar_tensor_tensor(mg1, mg1, DECAY, w1d[:], ALU.mult, ALU.add)
                nc.vector.tensor_scalar(pd1[:], mg1, 1.0, None, ALU.is_ge)
                w0d = ps.tile([128, AC], F32, tag="ps")
                mm_1024(w0d[:], W["wg0"], lambda kc: pd1[:, BC * kc: BC * (kc + 1)])
                nc.vector.scalar_tensor_tensor(mg0, mg0, DECAY, w0d[:], ALU.mult, ALU.add)
                if i == n_pos - 1:
                    nc.vector.scalar_tensor_tensor(
                        FACC[:], mg0, 1.0, FACC[:], ALU.is_ge, ALU.add)
                nc.vector.scalar_tensor_tensor(MGK[:, 0:AC], mg0, 1.0, mg0,
                                               ALU.is_lt, ALU.mult)

                # ---- off-chain (lower priority; gpsimd/scalar where possible) ----
                nc.vector.scalar_tensor_tensor(MGK[:, AC:2 * AC], mg1, 1.0, mg1,
                                               ALU.is_lt, ALU.mult)
                if i < n_pos - 1:
                    up_joint(i + 1)
                nc.scalar.copy(g1up_i, w1d[:])
                # gpsimd has no scalar_tensor_tensor opcode; use ts+tt pairs
                su0 = tmp.tile([128, AC], BF16, tag="su0")
                km0 = tmp.tile([128, AC], BF16, tag="km0")
                km1 = tmp.tile([128, AC], BF16, tag="km1")
                nc.gpsimd.tensor_scalar(su0[:], mi0, 1.0, None, ALU.is_ge)
                nc.gpsimd.tensor_tensor(s0_i, s0_i, su0[:], ALU.add)
                nc.gpsimd.tensor_scalar(km0[:], mi0, 1.0, None, ALU.is_lt)
                nc.gpsimd.tensor_tensor(mi0, mi0, km0[:], ALU.mult)
                nc.gpsimd.tensor_scalar(km1[:], mi1, 1.0, None, ALU.is_lt)
                nc.gpsimd.tensor_tensor(mi1, mi1, km1[:], ALU.mult)
                nc.vector.scalar_tensor_tensor(ME[:], ME[:], 1.0, ME[:], ALU.is_lt, ALU.mult)
                nc.vector.scalar_tensor_tensor(ME[:], ME[:], DECAY, ein_j, ALU.mult, ALU.add)

            # prologue for t=0, position 0
            nc.vector.scalar_tensor_tensor(ME[:], ME[:], DECAY, EIN[:, 0:AC],
                                           ALU.mult, ALU.add)
            up_joint(0)

            with tc.For_i(0, t_steps, 1) as _t:
                for i in range(n_pos):
                    pos_body(i)

                # fold per-position err sums
                tred = tmp.tile([128, 1], F32, tag="tred")
                nc.vector.tensor_reduce(tred[:], ESUM[:], mybir.AxisListType.X, ALU.add)
                nc.vector.tensor_add(EACC[:], EACC[:], tred[:])

                # batched g0up recompute for next t: state0 @ gen_w0.T
                for oc in range(KC):
                    pz = ps.tile([128, n_pos * BC], F32, tag="ps")
                    for kc in range(KC):
                        nc.tensor.matmul(
                            pz[:],
                            W["wg0"][:, kc * DM + oc * 128: kc * DM + (oc + 1) * 128],
                            s04[:, :, kc, :],
                            start=(kc == 0),
                            stop=(kc == KC - 1),
                        )
                    nc.scalar.copy(
                        g015[:, :, 0, oc, :],
                        pz[:].rearrange("p (i b) -> p i b", b=BC))
                # prep for position 0 of the next time step
                up_joint(0)

            # ---- Phase 3: outputs ----
            nc.sync.dma_start(facc_d[:], FACC[:])
            pe = ps.tile([1, 1], F32, tag="ps")
            nc.tensor.matmul(pe[:], EACC[:], ONES[:], start=True, stop=True)
            nc.vector.tensor_copy(ERRT[:], pe[:])
            nc.sync.dma_start(errsum_d[:], ERRT[:])

    nc.compile()
    return nc


def _build_logits_program():
    """General path (unused when final_act == 0): logits strip per core.
    Inputs: x32 (128, 32*KC) bf16 = final_act.T feature-major (col=32*kc+b);
    owt (128, KC*4096) bf16 = padded out_w.T strip tiles. Output (32, 4096)."""
    nc = bacc.Bacc(None, target_bir_lowering=False)
    OV = 4096
    x_d = nc.dram_tensor("x32", [128, 32 * KC], BF16, kind="ExternalInput")
    w_d = nc.dram_tensor("owt", [128, KC * OV], BF16, kind="ExternalInput")
    o_d = nc.dram_tensor("lg", [32, OV], F32, kind="ExternalOutput")
    with tile.TileContext(nc) as tc:
        with (
            tc.tile_pool(name="p", bufs=1) as pool,
            tc.tile_pool(name="ps", bufs=8, space="PSUM") as ps,
        ):
            X = pool.tile([128, 32 * KC], BF16, tag="x")
            WT = pool.tile([128, KC * OV], BF16, tag="w")
            OUT = pool.tile([32, OV], F32, tag="o")
            nc.sync.dma_start(X[:], x_d[:])
            nc.sync.dma_start(WT[:], w_d[:])
            for ob in range(OV // 512):
                pz = ps.tile([32, 512], F32, tag="ps")
                for kc in range(KC):
                    nc.tensor.matmul(
                        pz[:],
                        X[:, 32 * kc: 32 * (kc + 1)],
                        WT[:, kc * OV + ob * 512: kc * OV + (ob + 1) * 512],
                        start=(kc == 0),
                        stop=(kc == KC - 1),
                    )
                nc.vector.tensor_copy(OUT[:, ob * 512:(ob + 1) * 512], pz[:])
            nc.sync.dma_start(o_d[:], OUT[:])
    nc.compile()
    return nc


def _scan_inputs_for_core(c, input_ids, emb, enc_w, gen_w, inf_w):
    ids = np.asarray(input_ids)[c * BC:(c + 1) * BC, :]        # (BC, S)
    rows = np.asarray(emb, np.float32)[ids]                    # (BC, S, DM)
    wdt = ml_dtypes.float8_e4m3 if FP8_WEIGHTS else ml_dtypes.bfloat16
    return {
        "xemb": _pack_x(rows),
        "wenc": _pack_w(np.asarray(enc_w, np.float32).T),
        "wg0": _pack_w(np.asarray(gen_w[0], np.float32).T, wdt),
        "wg1": _pack_w(np.asarray(gen_w[1], np.float32).T, wdt),
        "wi0": _pack_w(np.asarray(inf_w[0], np.float32).T, wdt),
        "wi1": _pack_w(np.asarray(inf_w[1], np.float32).T, wdt),
    }


def kernel(input_ids, emb, enc_w, enc_b, gen_w, gen_b, inf_w, inf_b, out_w, out_b):
    global last_exec_time_ns
    ids_dtype = np.asarray(input_ids).dtype

    zero_bias = (not np.any(enc_b)) and (not np.any(gen_b)) and (not np.any(inf_b))
    assert zero_bias, "kernel implements the zero-bias fast path only"

    if "scan" not in _prog_cache:
        _prog_cache["scan"] = _build_scan_program()
    nc = _prog_cache["scan"]

    in_maps = [
        _scan_inputs_for_core(c, input_ids, emb, enc_w, gen_w, inf_w)
        for c in range(NCORE)
    ]
    res = run_bass_kernel_spmd(nc, in_maps, core_ids=list(range(NCORE)))
    last_exec_time_ns = res.exec_time_ns

    fa = []
    err_tot = 0.0
    for c in range(NCORE):
        f = res.results[c]["facc"]                              # (128, AC)
        fa.append(f.reshape(128, KC, BC).transpose(2, 1, 0).reshape(BC, DM))
        err_tot += float(res.results[c]["errsum"][0, 0])
    final_act = (np.concatenate(fa, axis=0) / T).astype(np.float32)   # (B, DM)
    spikes = np.float32(err_tot / (T * S * B))

    out_b = np.asarray(out_b, np.float32)
    if not np.any(final_act):
        # exact algebraic shortcut: 0 @ out_w.T == 0
        logits = np.broadcast_to(out_b, (B, V)).copy().astype(np.float32)
    else:
        logits = _logits_on_device(final_act, out_w, out_b)
    _ = ids_dtype
    return logits, spikes


def _logits_on_device(final_act, out_w, out_b):
    if "logits" not in _prog_cache:
        _prog_cache["logits"] = _build_logits_program()
    nc = _prog_cache["logits"]
    OV = 4096
    # x32: col = 32*kc + b  <-  final_act[b, 128*kc + p]
    arr = final_act.reshape(B, KC, 128).transpose(2, 1, 0)      # (p, kc, b)
    x32 = _bf16(np.ascontiguousarray(arr).reshape(128, KC * B))
    wt = np.asarray(out_w, np.float32)                          # (V, DM)
    in_maps = []
    for c in range(NCORE):
        strip = np.zeros((OV, DM), np.float32)
        lo, hi = c * (V // NCORE), (c + 1) * (V // NCORE)
        strip[: hi - lo] = wt[lo:hi]
        arrw = np.ascontiguousarray(strip.T).reshape(KC, 128, OV)
        wpack = _bf16(arrw.transpose(1, 0, 2).reshape(128, KC * OV))
        in_maps.append({"x32": x32, "owt": wpack})
    res = run_bass_kernel_spmd(nc, in_maps, core_ids=list(range(NCORE)))
    logits = np.concatenate(
        [res.results[c]["lg"][:, : V // NCORE] for c in range(NCORE)], axis=1)
    return (logits + out_b).astype(np.float32)
